# revision 1
# baseline (speedup 1.0000x reference)
"""Trainium2 Bass kernel for nn_AttentionBlock (B=8, N=2048, D=E=512).

Data-parallel over batch: each of the 8 NeuronCores computes one batch
element end-to-end (projection weights replicated); no collectives.

Numerics (rel err ~1.07e-2 vs fp32 reference; tolerance 2e-2):
  - ALL matmuls run in fp8e4 (TRN e4m3, max normal 240) with
    perf_mode=DoubleRow: both operands interleaved [K,2,free] so one
    pass contracts 256 — ~2x fp32r PE throughput.
  - The V projection, whose quantization error lands linearly in the
    output, uses a double-residual expansion to recover near-fp16
    accuracy from fp8 hardware:
        V = relu(x_hi@Wv_hi + x_lo@Wv_hi + x_hi@Wv_lo)
    with x_hi = fp8(x), x_lo = fp8(x - x_hi), same for Wv. This also
    removes the 4 MB fp32 x^T load the fp32r V-path needed.
  - The softmax scale 1/sqrt(512) and a constant -3 shift are folded
    into the EXP activation (exp(s*S-3)), keeping Q/K at natural
    magnitude and exp(S) within fp8e4 range [~0.2, 55]; row sums are
    taken over the SAME fp8 P values so the shift cancels exactly in
    the normalization.

Schedule per core (PE order):
  - QK projections: DoubleRow stationaries reused across 4 q-chunks.
  - fused phase: scores kt-outer over a q-half (stationary KT slice
    reused 2x), one merged EXP per kt over a 2-bank PSUM pair, V-proj
    nt-groups interleaved into the ACT-bound stream (relu+fp8 cast on
    DVE).
  - AV: row-sum matmuls against a ones stationary, V-slice stationaries
    reused 2x, normalization via DVE reciprocal -> K=1 ones matmul
    partition-broadcast -> ACT copy -> DVE multiply.
  - every per-iteration tensor moves in ONE large dma_start (~2us fixed
    cost each on TRN2), spread across the SP HWDGE ring and SWDGE.
"""

import sys

if "/opt/trn_rl_repo" not in sys.path:
    sys.path.insert(0, "/opt/trn_rl_repo")

from contextlib import ExitStack, nullcontext

import ml_dtypes
import numpy as np

import concourse.bacc as bacc
import concourse.tile as tile
from concourse import mybir
from concourse.bass_utils import run_bass_kernel_spmd

F32 = mybir.dt.float32
F32R = mybir.dt.float32r
F8 = mybir.dt.float8e4
U8 = mybir.dt.uint8
RELU = mybir.ActivationFunctionType.Relu
EXP = mybir.ActivationFunctionType.Exp
COPY = mybir.ActivationFunctionType.Copy
DR = mybir.MatmulPerfMode.DoubleRow

B = 8
N = 2048
D = 512
E = 512
P = 128
NT = N // P
DT = D // P
ET = E // P
HP = D // (2 * P)
KP = N // (2 * P)
QCW = 512
QC = N // QCW
SCALE = 1.0 / float(np.sqrt(E))
EXP_SHIFT = -3.0


def _build_nc(v_bias: bool = True, n_iters: int = 1):
    nc = bacc.Bacc("TRN2", num_devices=1)

    # batched inputs: partition-major so each is a single contiguous DMA
    x8d = nc.dram_tensor("x8a", [P, HP, 2, N], U8, kind="ExternalInput").ap()
    wqk8d = nc.dram_tensor("wqk8", [P, HP, 2, 2 * E], U8, kind="ExternalInput").ap()
    xlod = nc.dram_tensor("x8lo", [P, HP, 2, N], U8, kind="ExternalInput").ap()
    wv8d = nc.dram_tensor("wv8", [P, HP, 2, 2 * E], U8, kind="ExternalInput").ap()
    bqkd = nc.dram_tensor("bqk", [P, 2 * ET], F32, kind="ExternalInput").ap()
    bvd = nc.dram_tensor("bv", [E], F32, kind="ExternalInput").ap()
    oT = nc.dram_tensor("oT", [E, N], F32, kind="ExternalOutput").ap()

    ones8_np = np.full((P, 2, 16), 0x38, np.uint8)  # 1.0 in e4m3
    ones_dram = nc.inline_tensor(ones8_np, name="ones8")
    shift_dram = nc.inline_tensor(
        np.full((P, 1), EXP_SHIFT, np.float32), name="eshift"
    )
    ones_r_dram = nc.inline_tensor(np.ones((1, P), np.float32), name="ones_r")

    with tile.TileContext(nc) as tc:
        with ExitStack() as ctx:
            sing = ctx.enter_context(tc.tile_pool(name="singles", bufs=1))
            data = ctx.enter_context(tc.tile_pool(name="data", bufs=1))

            ones8 = sing.tile([P, 2, 16], F8)
            nc.sync.dma_start(out=ones8, in_=ones_dram.ap().bitcast(F8))
            eshift = sing.tile([P, 1], F32)
            nc.sync.dma_start(out=eshift, in_=shift_dram.ap())
            ones_row = sing.tile([1, P], F32R)
            nc.sync.dma_start(out=ones_row, in_=ones_r_dram.ap().bitcast(F32R))

            x8a = data.tile([P, HP, 2, N], F8, tag="x8a")
            wqk8 = data.tile([P, HP, 2, 2 * E], F8, tag="wqk8")
            x8lo = data.tile([P, HP, 2, N], F8, tag="x8lo")
            wv8 = data.tile([P, HP, 2, 2 * E], F8, tag="wv8")
            qt8 = [data.tile([P, 2, N], F8, name=f"qt8_{h}", tag=f"qt8_{h}") for h in range(HP)]
            kt8 = [data.tile([P, 2, N], F8, name=f"kt8_{h}", tag=f"kt8_{h}") for h in range(HP)]
            v8 = data.tile([P, KP, 2, E], F8, tag="v8")
            ptall = data.tile([P, QC, KP, 2, QCW], F8, tag="ptall")
            bqk_sb = data.tile([P, 2 * ET], F32, tag="bqk")
            bv_sb = data.tile([1, E], F32R, tag="bv")

            loop_cm = tc.For_i(0, n_iters) if n_iters > 1 else nullcontext()
            ctx.enter_context(loop_cm)

            # SP ring: wqk8 then xta; SWDGE: x8a then wva — the PE can
            # start QK-proj at ~6us (wqk8+x8a), V-proj unblocks at ~16us.
            nc.sync.dma_start(out=wqk8, in_=wqk8d.bitcast(F8))
            nc.gpsimd.dma_start(out=x8a, in_=x8d.bitcast(F8))
            nc.sync.dma_start(out=bqk_sb, in_=bqkd)
            nc.sync.dma_start(out=wv8, in_=wv8d.bitcast(F8))
            nc.gpsimd.dma_start(out=x8lo, in_=xlod.bitcast(F8))
            nc.sync.dma_start(out=bv_sb, in_=bvd.unsqueeze(0).bitcast(F32R))

            # ---- QK projections: fp8 DR, stationary reused 4x ----
            with tc.tile_pool(name="psA", bufs=8, space="PSUM") as psA:
                for wi, dst in ((0, qt8), (1, kt8)):
                    for et in range(ET):
                        ps = [
                            psA.tile([P, QCW], F32, name=f"ps1_{qq}", tag="ps1")
                            for qq in range(QC)
                        ]
                        for hp in range(HP):
                            for qc in range(QC):
                                nc.tensor.matmul(
                                    ps[qc],
                                    lhsT=wqk8[:, hp, :, wi * E + et * P:wi * E + (et + 1) * P],
                                    rhs=x8a[:, hp, :, qc * QCW:(qc + 1) * QCW],
                                    start=(hp == 0),
                                    stop=(hp == HP - 1),
                                    perf_mode=DR,
                                )
                        for qc in range(QC):
                            nc.scalar.activation(
                                out=dst[et // 2][:, et % 2, qc * QCW:(qc + 1) * QCW],
                                in_=ps[qc],
                                func=RELU,
                                bias=bqk_sb[:, wi * ET + et:wi * ET + et + 1],
                            )

            # ---- fused scores (kt-outer, q-half) + V projection ----
            with (
                tc.tile_pool(name="psS", bufs=2, space="PSUM") as psS,
                tc.tile_pool(name="psV", bufs=2, space="PSUM") as psV,
            ):
                def v_proj_nt(nt):
                    # V = relu((x_hi+x_lo)@Wv_hi + x_hi@Wv_lo), all fp8 DR
                    ps = psV.tile([P, E], F32, tag="psv")
                    if v_bias:
                        nc.tensor.matmul(
                            ps, lhsT=ones_row, rhs=bv_sb, start=True, stop=False
                        )
                    terms = (
                        (x8a, 0),    # x_hi @ Wv_hi
                        (x8lo, 0),   # x_lo @ Wv_hi
                        (x8a, E),    # x_hi @ Wv_lo
                    )
                    first = 0 if v_bias else 1
                    for ti, (xs, woff) in enumerate(terms):
                        for hp in range(HP):
                            nc.tensor.matmul(
                                ps,
                                lhsT=xs[:, hp, :, nt * P:(nt + 1) * P],
                                rhs=wv8[:, hp, :, woff:woff + E],
                                start=(ti == 0 and hp == 0 and not v_bias),
                                stop=(ti == 2 and hp == HP - 1),
                                perf_mode=DR,
                            )
                    nc.vector.tensor_scalar_max(v8[:, nt // 2, nt % 2, :], ps, 0.0)

                for h in range(2):
                    qcs = (2 * h, 2 * h + 1)
                    for kt in range(NT):
                        ps = psS.tile([P, 2, QCW], F32, tag="psS")
                        for hp in range(HP):
                            for j, qc in enumerate(qcs):
                                nc.tensor.matmul(
                                    ps[:, j, :],
                                    lhsT=kt8[hp][:, :, kt * P:(kt + 1) * P],
                                    rhs=qt8[hp][:, :, qc * QCW:(qc + 1) * QCW],
                                    start=(hp == 0),
                                    stop=(hp == HP - 1),
                                    perf_mode=DR,
                                )
                        v_proj_nt(h * NT // 2 + kt // 2) if kt % 2 == 0 else None
                        nc.scalar.activation(
                            out=ptall[:, 2 * h:2 * h + 2, kt // 2, kt % 2, :],
                            in_=ps,
                            func=EXP,
                            scale=SCALE,
                            bias=eshift[:, 0:1],
                        )

            # ---- sums + AV + normalize ----
            with (
                tc.tile_pool(name="po", bufs=2, space="PSUM") as po_pool,
                tc.tile_pool(name="posum", bufs=2, space="PSUM") as posum_pool,
                tc.tile_pool(name="rbps", bufs=2, space="PSUM") as rb_pool,
                tc.tile_pool(name="otp", bufs=3) as ot_pool,
                tc.tile_pool(name="small", bufs=4) as small_pool,
            ):
                for h in range(2):
                    qcs = (2 * h, 2 * h + 1)
                    posum = {}
                    for qc in qcs:
                        posum[qc] = posum_pool.tile(
                            [1, QCW], F32, name=f"posum{qc}", tag="posum"
                        )
                        for kp in range(KP):
                            nc.tensor.matmul(
                                posum[qc],
                                lhsT=ones8[:, :, 0:1],
                                rhs=ptall[:, qc, kp, :, :],
                                start=(kp == 0),
                                stop=(kp == KP - 1),
                                perf_mode=DR,
                            )
                    rb = {}
                    for qc in qcs:
                        rinv = small_pool.tile([1, QCW], F32R, tag="rinv")
                        with nc.allow_low_precision(reason="f32r 1/rowsum bcast"):
                            nc.vector.reciprocal(out=rinv, in_=posum[qc])
                        rb_ps = rb_pool.tile([P, QCW], F32, tag="rbps")
                        nc.tensor.matmul(
                            rb_ps, lhsT=ones_row, rhs=rinv, start=True, stop=True
                        )
                        rb[qc] = small_pool.tile([P, QCW], F32, name=f"rb{qc}", tag="rb")
                        nc.scalar.activation(out=rb[qc], in_=rb_ps, func=COPY)
                    for et in range(ET):
                        po = [
                            po_pool.tile([P, QCW], F32, name=f"po_{j}", tag="po")
                            for j in range(2)
                        ]
                        for kp in range(KP):
                            for j, qc in enumerate(qcs):
                                nc.tensor.matmul(
                                    po[j],
                                    lhsT=v8[:, kp, :, et * P:(et + 1) * P],
                                    rhs=ptall[:, qc, kp, :, :],
                                    start=(kp == 0),
                                    stop=(kp == KP - 1),
                                    perf_mode=DR,
                                )
                        ot = ot_pool.tile([P, 2, QCW], F32, tag="ot")
                        for j, qc in enumerate(qcs):
                            nc.vector.tensor_mul(ot[:, j, :], po[j], rb[qc])
                        nc.sync.dma_start(
                            out=oT[et * P:(et + 1) * P, 2 * h * QCW:(2 * h + 2) * QCW],
                            in_=ot,
                        )

    nc.compile()
    return nc


def build_nc(n_iters: int = 1, v_bias: bool = False):
    return _build_nc(v_bias=v_bias, n_iters=n_iters)


# ---------------- host-side packing ----------------

F8NP = ml_dtypes.float8_e4m3


def _to_f8_u8(a):
    return np.clip(a, -240, 240).astype(F8NP).view(np.uint8)


def _pack_p(m):
    """[D, cols] -> [128p, 2hp, 2i, cols] d-interleaved fp8 (uint8)."""
    r = m.reshape(2, 2, 128, m.shape[1]).transpose(0, 2, 1, 3)
    return np.ascontiguousarray(_to_f8_u8(r).transpose(1, 0, 2, 3))


def make_in_maps(inputs):
    x = np.asarray(inputs["x"], dtype=np.float32)
    Wq = np.ascontiguousarray(inputs["Wq"], dtype=np.float32)
    Wk = np.ascontiguousarray(inputs["Wk"], dtype=np.float32)
    Wv = np.ascontiguousarray(inputs["Wv"], dtype=np.float32)
    bq = np.asarray(inputs["bq"], dtype=np.float32)
    bk = np.asarray(inputs["bk"], dtype=np.float32)
    bv = np.ascontiguousarray(inputs["bv"], dtype=np.float32)

    wvhi = np.clip(Wv, -240, 240).astype(F8NP)
    wvlo = Wv - wvhi.astype(np.float32)
    wqk8 = np.ascontiguousarray(
        np.concatenate([_pack_p(Wq), _pack_p(Wk)], axis=3)
    )
    wv8 = np.ascontiguousarray(
        np.concatenate([_pack_p(Wv), _pack_p(wvlo)], axis=3)
    )
    bqk = np.ascontiguousarray(
        np.concatenate([bq.reshape(4, 128).T, bk.reshape(4, 128).T], axis=1)
    )

    in_maps = []
    for c in range(B):
        xT = np.ascontiguousarray(x[c].T)
        xhi = np.clip(xT, -240, 240).astype(F8NP)
        xlo = xT - xhi.astype(np.float32)
        in_maps.append({
            "x8a": _pack_p(xT),
            "x8lo": _pack_p(xlo),
            "wqk8": wqk8,
            "wv8": wv8,
            "bqk": bqk,
            "bv": bv,
        })
    return in_maps


_NC_CACHE = {}


def kernel(**inputs) -> np.ndarray:
    v_bias = bool(np.any(np.asarray(inputs["bv"])))
    if v_bias not in _NC_CACHE:
        _NC_CACHE[v_bias] = _build_nc(v_bias=v_bias)
    nc = _NC_CACHE[v_bias]

    in_maps = make_in_maps(inputs)
    res = run_bass_kernel_spmd(nc, in_maps, core_ids=list(range(B)))
    out = np.stack(
        [np.ascontiguousarray(res.results[c]["oT"].T) for c in range(B)]
    )
    return out.astype(np.float32)

